# revision 13
# baseline (speedup 1.0000x reference)
"""GAT layer (DGL GATConv + BatchNorm + ELU + residual) on 8 Trainium2 cores.

Strategy (dst-sharded graph parallel):
  - Sort edges by destination; shard destination nodes across 8 cores
    (12544 slots/core = 98 blocks x 128 slots, load-balanced by degree).
  - Each core builds the full node table  [feat | el] = [x@W | x@W@almat]
    (100353 rows x 136 f32; row 100352 is a sentinel with el=-1e30) in its
    own HBM, then gathers 544B rows per edge with indirect DMA.
  - Per 128-edge tile: one-hot S (edge->slot) built on DVE via is_equal;
    er expanded edge-wise with a one-hot matmul; softmax without max
    subtraction (exp range is tiny); messages scaled by w=exp(lrelu(e));
    scatter-reduce into PSUM via S^T @ msg on the PE.
  - BatchNorm batch stats are global: launch 1 returns per-core partial
    sums, the host reduces 2x128 floats, launch 2 applies the affine fold
    a*h+c, ELU and the residual in channel-major layout.
"""
import sys
sys.path.insert(0, "/opt/trn_rl_repo")
import numpy as np

import concourse.bass as bass
import concourse.bacc as bacc
import concourse.mybir as mybir
import concourse.tile as tile
from concourse.bass_utils import run_bass_kernel_spmd

F32 = mybir.dt.float32
F16 = mybir.dt.float16
I32 = mybir.dt.int32

N = 100000
E = 1600000
IN_DIM = 128
H = 8
D = 16
HD = 128
NCORES = 8
NBLK = 98                 # blocks per core
TPB = 17                  # tiles per block
SLOTS = NBLK * 128        # 12544 slots per core
TILES = NBLK * TPB        # 1666 tiles per core
EDGES_PAD = TILES * 128   # padded edge slots per core
NTOT = NCORES * SLOTS     # 100352 padded node count
SENT = NTOT               # sentinel table row
ROW = IN_DIM + H          # 136 psum row (feat | el)
ROW16 = IN_DIM + 2 * H    # 144 fp16 slots per table row (el stored as fp32 pairs)
NEG_SLOPE = 0.2
EPS = 1e-5

LAST_EXEC_NS = [0, 0]

_cache = {}


def _build_launch1():
    nc = bacc.Bacc("TRN2", target_bir_lowering=False, debug=False,
                   num_devices=NCORES)
    xTh = nc.dram_tensor("xTh", [128, NTOT], F16, kind="ExternalInput")
    xTl = nc.dram_tensor("xTl", [128, NTOT], F16, kind="ExternalInput")
    xTp = nc.dram_tensor("xTp", [128, SLOTS], F32, kind="ExternalInput")
    Wd = nc.dram_tensor("W", [IN_DIM, HD], F32, kind="ExternalInput")
    amd = nc.dram_tensor("am", [HD, 2 * H], F32, kind="ExternalInput")
    iota_r = nc.dram_tensor("iota_r", [128, 128], F16, kind="ExternalInput")
    iota_c = nc.dram_tensor("iota_c", [128, 1], F32, kind="ExternalInput")
    srcd = nc.dram_tensor("srci", [128, TILES], I32, kind="ExternalInput")
    dslotd = nc.dram_tensor("dslot", [128, TILES], F32, kind="ExternalInput")
    drowd = nc.dram_tensor("drow", [1, EDGES_PAD], F16, kind="ExternalInput")

    h_out = nc.dram_tensor("h_out", [SLOTS, HD], F32, kind="ExternalOutput")
    st_out = nc.dram_tensor("st_out", [128, 2], F32, kind="ExternalOutput")
    table = nc.dram_tensor("table", [NTOT + 1, ROW16], F16)

    NT_A = NTOT // 128  # 784 node tiles for table build

    with tile.TileContext(nc) as tc:
        with (
            tc.tile_pool(name="const", bufs=1) as constp,
            tc.tile_pool(name="pa_sb", bufs=4) as pa_sb,
            tc.tile_pool(name="ers", bufs=1) as ersp,
            tc.tile_pool(name="g4p", bufs=10) as g4p,
            tc.tile_pool(name="sp", bufs=8) as sp,
            tc.tile_pool(name="st4p", bufs=5) as st4p,
            tc.tile_pool(name="wp", bufs=6) as wp,
            tc.tile_pool(name="drp", bufs=3) as drp,
            tc.tile_pool(name="fin", bufs=3) as finp,
        ):
            # ---- constants ----
            iota_row = constp.tile([128, 128], F16)
            nc.sync.dma_start(out=iota_row[:], in_=iota_r[:])
            iota_col = constp.tile([128, 1], F32)
            nc.sync.dma_start(out=iota_col[:], in_=iota_c[:])
            ones_row = constp.tile([1, 128], F16)
            nc.vector.memset(ones_row[:], 1.0)
            ones_col = constp.tile([128, 1], F32)
            nc.vector.memset(ones_col[:], 1.0)
            ones_col16 = constp.tile([128, 1], F16)
            nc.vector.memset(ones_col16[:], 1.0)

            pa_scope = tc.tile_pool(name="pa_ps", bufs=4, space="PSUM")
            pa_ps = pa_scope.__enter__()
            # ---- Wfull = [W | W@almat | W@armat]  [128, 144] ----
            W_sb = constp.tile([128, HD], F32)
            nc.sync.dma_start(out=W_sb[:], in_=Wd[:])
            am_sb = constp.tile([128, 2 * H], F32)
            nc.sync.dma_start(out=am_sb[:], in_=amd[:])
            ident = constp.tile([128, 128], F32)
            from concourse.masks import make_identity
            make_identity(nc, ident[:])
            wt_ps = pa_ps.tile([128, 128], F32, tag="pa")
            nc.tensor.transpose(out=wt_ps[:], in_=W_sb[:], identity=ident[:])
            WT_sb = constp.tile([128, 128], F32)
            nc.vector.tensor_copy(out=WT_sb[:], in_=wt_ps[:])
            Wfull = constp.tile([128, IN_DIM + 2 * H], F32)
            nc.vector.tensor_copy(out=Wfull[:, 0:HD], in_=W_sb[:])
            wlr_ps = pa_ps.tile([128, 2 * H], F32, tag="pa")
            nc.tensor.matmul(out=wlr_ps[:], lhsT=WT_sb[:], rhs=am_sb[:],
                             start=True, stop=True)
            nc.vector.tensor_copy(out=Wfull[:, HD:HD + 2 * H], in_=wlr_ps[:])
            Wh = constp.tile([128, IN_DIM + 2 * H], F16)
            nc.vector.tensor_copy(out=Wh[:], in_=Wfull[:])
            Wh32 = constp.tile([128, IN_DIM + 2 * H], F32)
            nc.vector.tensor_copy(out=Wh32[:], in_=Wh[:])
            Wl = constp.tile([128, IN_DIM + 2 * H], F16)
            nc.vector.tensor_tensor(out=Wl[:], in0=Wfull[:], in1=Wh32[:],
                                    op=mybir.AluOpType.subtract)

            # ---- sentinel row ----
            sent_sb = constp.tile([1, ROW16], F16)
            nc.vector.memset(sent_sb[:], 0.0)
            nc.vector.memset(sent_sb[:, IN_DIM:ROW16].bitcast(F32), -1e30)
            nc.sync.dma_start(out=table[SENT:SENT + 1, :], in_=sent_sb[:])

            # ---- phase A: full node table (groups of 4 tiles) ----
            for t4 in range(NT_A // 4):
                x4h = pa_sb.tile([128, 512], F16, tag="xth")
                nc.scalar.dma_start(out=x4h[:], in_=xTh[:, t4 * 512:(t4 + 1) * 512])
                x4l = pa_sb.tile([128, 512], F16, tag="xtl")
                nc.scalar.dma_start(out=x4l[:], in_=xTl[:, t4 * 512:(t4 + 1) * 512])
                row4 = pa_sb.tile([128, 4 * ROW16], F16, tag="row4")
                for k in range(4):
                    ps = pa_ps.tile([128, ROW], F32, tag="pa")
                    nc.tensor.matmul(out=ps[:], lhsT=x4h[:, k * 128:(k + 1) * 128],
                                     rhs=Wh[:, 0:ROW], start=True, stop=False)
                    nc.tensor.matmul(out=ps[:], lhsT=x4h[:, k * 128:(k + 1) * 128],
                                     rhs=Wl[:, 0:ROW], start=False, stop=False)
                    nc.tensor.matmul(out=ps[:], lhsT=x4l[:, k * 128:(k + 1) * 128],
                                     rhs=Wh[:, 0:ROW], start=False, stop=True)
                    o = k * ROW16
                    if k % 2 == 0:
                        nc.vector.tensor_copy(out=row4[:, o:o + IN_DIM],
                                              in_=ps[:, 0:IN_DIM])
                    else:
                        nc.scalar.activation(row4[:, o:o + IN_DIM],
                                             ps[:, 0:IN_DIM],
                                             mybir.ActivationFunctionType.Copy)
                    nc.vector.tensor_copy(
                        out=row4[:, o + IN_DIM:o + ROW16].bitcast(F32),
                        in_=ps[:, IN_DIM:ROW])
                nc.sync.dma_start(
                    out=table[t4 * 512:(t4 + 1) * 512, :].rearrange(
                        "(f p) c -> p f c", f=4),
                    in_=row4[:].rearrange("p (f c) -> p f c", c=ROW16))

            # ---- er for own slots: hi/lo fp16 pairs [128, 98*16] ----
            er_sb = ersp.tile([128, NBLK * 2 * H], F16)
            for b in range(NBLK):
                xp_sb = pa_sb.tile([128, 128], F32, tag="xp")
                nc.scalar.dma_start(out=xp_sb[:], in_=xTp[:, b * 128:(b + 1) * 128])
                ps = pa_ps.tile([128, H], F32, tag="pa")
                nc.tensor.matmul(out=ps[:], lhsT=xp_sb[:],
                                 rhs=Wfull[:, ROW:ROW + H], start=True, stop=True)
                o = b * 2 * H
                nc.vector.tensor_copy(out=er_sb[:, o:o + H], in_=ps[:])
                hi32 = finp.tile([128, H], F32, tag="hi32")
                nc.vector.tensor_copy(out=hi32[:], in_=er_sb[:, o:o + H])
                nc.vector.tensor_tensor(out=er_sb[:, o + H:o + 2 * H],
                                        in0=ps[:], in1=hi32[:],
                                        op=mybir.AluOpType.subtract)

            pa_scope.__exit__(None, None, None)
            blk_scope = tc.tile_pool(name="blk_ps", bufs=2, space="PSUM")
            blk_ps = blk_scope.__enter__()
            erp_scope = tc.tile_pool(name="er_ps", bufs=2, space="PSUM")
            er_ps = erp_scope.__enter__()
            dt_scope = tc.tile_pool(name="dt_ps", bufs=2, space="PSUM")
            dt_ps = dt_scope.__enter__()
            st_scope = tc.tile_pool(name="stat_ps", bufs=1, space="PSUM")
            stat_ps = st_scope.__enter__()
            # ---- index preloads ----
            src_sb = constp.tile([128, TILES], I32)
            nc.sync.dma_start(out=src_sb[:], in_=srcd[:])
            dslot_sb = constp.tile([128, TILES], F32)
            nc.sync.dma_start(out=dslot_sb[:], in_=dslotd[:])

            # ---- stats accumulators (persist across blocks) ----
            s1_ps = stat_ps.tile([128, 1], F32)
            s2_ps = stat_ps.tile([128, 1], F32)

            GPB = TPB // 4 + (1 if TPB % 4 else 0)  # groups per block (of <=4 tiles)

            # ---- phase B ----
            for b in range(NBLK):
                dr = drp.tile([1, TPB * 128], F16, tag="dr")
                nc.sync.dma_start(out=dr[:],
                                  in_=drowd[:, b * TPB * 128:(b + 1) * TPB * 128])
                psb = blk_ps.tile([128, ROW], F32, tag="blk")
                for g in range(GPB):
                    t0 = g * 4
                    nt = min(4, TPB - t0)
                    ne = nt * 128
                    # replicate dst slots across partitions, build ST
                    dtp = dt_ps.tile([128, 512], F32, tag="dt")
                    nc.tensor.matmul(out=dtp[:, :ne], lhsT=ones_row[:],
                                     rhs=dr[:, t0 * 128:t0 * 128 + ne],
                                     start=True, stop=True)
                    st4 = st4p.tile([128, 512], F16, tag="st4")
                    nc.vector.tensor_scalar(out=st4[:, :ne], in0=dtp[:, :ne],
                                            scalar1=iota_col[:],
                                            scalar2=None,
                                            op0=mybir.AluOpType.is_equal)
                    # gather 4 tiles worth of table rows
                    g4 = g4p.tile([128, 4 * ROW16], F16, tag="g4")
                    for k in range(nt):
                        col = b * TPB + t0 + k
                        nc.gpsimd.indirect_dma_start(
                            out=g4[:, k * ROW16:(k + 1) * ROW16],
                            out_offset=None,
                            in_=table[:],
                            in_offset=bass.IndirectOffsetOnAxis(
                                ap=src_sb[:, col:col + 1], axis=0),
                        )
                    # er per edge via one-hot matmul
                    erp = er_ps.tile([128, 4 * 2 * H], F32, tag="erp")
                    for k in range(nt):
                        nc.tensor.matmul(
                            out=erp[:, k * 2 * H:(k + 1) * 2 * H],
                            lhsT=st4[:, k * 128:(k + 1) * 128],
                            rhs=er_sb[:, b * 2 * H:(b + 1) * 2 * H],
                            start=True, stop=True)
                    # e = el + er ; w = exp(lrelu(e))
                    wsb = wp.tile([128, 4 * H], F32, tag="w")
                    el_view = (g4[:].rearrange("p (t c) -> p t c", c=ROW16)
                               [:, 0:nt, IN_DIM:ROW16].bitcast(F32))
                    erp_v = erp[:, :nt * 2 * H].rearrange("p (t u) -> p t u", u=2 * H)
                    w_v = wsb[:, :nt * H].rearrange("p (t h) -> p t h", h=H)
                    nc.vector.tensor_tensor(
                        out=w_v, in0=el_view, in1=erp_v[:, :, 0:H],
                        op=mybir.AluOpType.add)
                    nc.vector.tensor_tensor(
                        out=w_v, in0=w_v, in1=erp_v[:, :, H:2 * H],
                        op=mybir.AluOpType.add)
                    w5 = wp.tile([128, 4 * H], F32, tag="w5")
                    nc.vector.tensor_scalar(out=w5[:, :nt * H],
                                            in0=wsb[:, :nt * H],
                                            scalar1=NEG_SLOPE, scalar2=None,
                                            op0=mybir.AluOpType.mult)
                    nc.vector.tensor_tensor(out=wsb[:, :nt * H],
                                            in0=wsb[:, :nt * H],
                                            in1=w5[:, :nt * H],
                                            op=mybir.AluOpType.max)
                    nc.scalar.activation(wsb[:, :nt * H], wsb[:, :nt * H],
                                         mybir.ActivationFunctionType.Exp)
                    # w into fp16 slots 128:136; scale messages
                    g4r = g4[:].rearrange("p (t c) -> p t c", c=ROW16)
                    w16_view = g4r[:, 0:nt, IN_DIM:IN_DIM + H]
                    nc.vector.tensor_copy(
                        out=w16_view,
                        in_=wsb[:, :nt * H].rearrange("p (t h) -> p t h", h=H))
                    feat_view = g4r[:, 0:nt, 0:IN_DIM]
                    w_b = (w16_view
                           .rearrange("p t (h one) -> p t h one", h=H, one=1)
                           .to_broadcast([128, nt, H, D]))
                    nc.vector.tensor_tensor(
                        out=feat_view.rearrange("p t (h d) -> p t h d", d=D),
                        in0=feat_view.rearrange("p t (h d) -> p t h d", d=D),
                        in1=w_b,
                        op=mybir.AluOpType.mult)
                    # per-tile one-hot S + scatter matmul
                    for k in range(nt):
                        col = b * TPB + t0 + k
                        s_sb = sp.tile([128, 128], F16, tag="s")
                        nc.vector.tensor_scalar(
                            out=s_sb[:], in0=iota_row[:],
                            scalar1=dslot_sb[:, col:col + 1],
                            scalar2=None,
                            op0=mybir.AluOpType.is_equal)
                        ti = t0 + k
                        nc.tensor.matmul(out=psb[:],
                                         lhsT=s_sb[:],
                                         rhs=g4[:, k * ROW16:k * ROW16 + ROW],
                                         start=(ti == 0), stop=(ti == TPB - 1))
                # ---- block finalize ----
                ssum = finp.tile([128, H], F32, tag="ssum")
                nc.vector.tensor_scalar(out=ssum[:], in0=psb[:, IN_DIM:ROW],
                                        scalar1=1e-30, scalar2=None,
                                        op0=mybir.AluOpType.add)
                rec = finp.tile([128, H], F32, tag="rec")
                nc.vector.reciprocal(out=rec[:], in_=ssum[:])
                h_sb = finp.tile([128, HD], F32, tag="h")
                rec_b = (rec[:].rearrange("p (h one) -> p h one", h=H, one=1)
                         .to_broadcast([128, H, D]))
                nc.vector.tensor_tensor(
                    out=h_sb[:].rearrange("p (h d) -> p h d", d=D),
                    in0=psb[:, 0:IN_DIM].rearrange("p (h d) -> p h d", d=D),
                    in1=rec_b, op=mybir.AluOpType.mult)
                h16 = finp.tile([128, HD], F16, tag="h16")
                nc.vector.tensor_copy(out=h16[:], in_=h_sb[:])
                sq_sb = finp.tile([128, HD], F16, tag="sq")
                nc.scalar.activation(sq_sb[:], h_sb[:],
                                     mybir.ActivationFunctionType.Square)
                nc.tensor.matmul(out=s1_ps[:], lhsT=h16[:], rhs=ones_col16[:],
                                 start=(b == 0), stop=(b == NBLK - 1))
                nc.tensor.matmul(out=s2_ps[:], lhsT=sq_sb[:], rhs=ones_col16[:],
                                 start=(b == 0), stop=(b == NBLK - 1))
                nc.sync.dma_start(out=h_out[b * 128:(b + 1) * 128, :], in_=h_sb[:])

            stat_sb = constp.tile([128, 2], F32)
            nc.vector.tensor_copy(out=stat_sb[:, 0:1], in_=s1_ps[:])
            nc.vector.tensor_copy(out=stat_sb[:, 1:2], in_=s2_ps[:])
            nc.sync.dma_start(out=st_out[:], in_=stat_sb[:])
            st_scope.__exit__(None, None, None)
            dt_scope.__exit__(None, None, None)
            erp_scope.__exit__(None, None, None)
            blk_scope.__exit__(None, None, None)

    nc.compile()
    return nc


def _build_launch2():
    nc = bacc.Bacc("TRN2", target_bir_lowering=False, debug=False,
                   num_devices=NCORES)
    h_in = nc.dram_tensor("h_in", [SLOTS, HD], F32, kind="ExternalInput")
    xTp = nc.dram_tensor("xTp", [128, SLOTS], F32, kind="ExternalInput")
    ac = nc.dram_tensor("ac", [128, 2], F32, kind="ExternalInput")
    out_t = nc.dram_tensor("out_t", [128, SLOTS], F32, kind="ExternalOutput")

    CH = 512
    NCH = SLOTS // CH  # 24.5 -> handle 24 full + 1 tail of 256
    chunks = [(i * CH, CH) for i in range(NCH)]
    if SLOTS % CH:
        chunks.append((NCH * CH, SLOTS % CH))

    with tile.TileContext(nc) as tc:
        with (
            tc.tile_pool(name="const", bufs=1) as constp,
            tc.tile_pool(name="ld", bufs=4) as ldp,
            tc.tile_pool(name="ps", bufs=3, space="PSUM") as psp,
            tc.tile_pool(name="wk", bufs=3) as wkp,
        ):
            from concourse.masks import make_identity
            ident = constp.tile([128, 128], F32)
            make_identity(nc, ident[:])
            ac_sb = constp.tile([128, 2], F32)
            nc.sync.dma_start(out=ac_sb[:], in_=ac[:])

            for (o, w) in chunks:
                nk = w // 128
                hp = psp.tile([128, CH], F32, tag="hp")
                for k in range(nk):
                    hl = ldp.tile([128, 128], F32, tag="hl")
                    nc.sync.dma_start(
                        out=hl[:], in_=h_in[o + k * 128:o + (k + 1) * 128, :])
                    nc.tensor.transpose(out=hp[:, k * 128:(k + 1) * 128],
                                        in_=hl[:], identity=ident[:])
                h2 = wkp.tile([128, CH], F32, tag="h2")
                nc.vector.tensor_scalar(out=h2[:, :w], in0=hp[:, :w],
                                        scalar1=ac_sb[:, 0:1],
                                        scalar2=ac_sb[:, 1:2],
                                        op0=mybir.AluOpType.mult,
                                        op1=mybir.AluOpType.add)
                m = wkp.tile([128, CH], F32, tag="m")
                nc.vector.tensor_scalar(out=m[:, :w], in0=h2[:, :w],
                                        scalar1=0.0, scalar2=None,
                                        op0=mybir.AluOpType.min)
                nc.scalar.activation(m[:, :w], m[:, :w],
                                     mybir.ActivationFunctionType.Exp)
                nc.vector.tensor_scalar(out=m[:, :w], in0=m[:, :w],
                                        scalar1=-1.0, scalar2=None,
                                        op0=mybir.AluOpType.add)
                # elu = max(h2, exp(min(h2,0))-1)
                nc.vector.tensor_tensor(out=h2[:, :w], in0=h2[:, :w],
                                        in1=m[:, :w],
                                        op=mybir.AluOpType.max)
                xt = ldp.tile([128, CH], F32, tag="xt")
                nc.sync.dma_start(out=xt[:, :w], in_=xTp[:, o:o + w])
                nc.vector.tensor_tensor(out=h2[:, :w], in0=h2[:, :w],
                                        in1=xt[:, :w], op=mybir.AluOpType.add)
                nc.sync.dma_start(out=out_t[:, o:o + w], in_=h2[:, :w])

    nc.compile()
    return nc


def _host_prep(x, src, dst):
    """Shard + balance + pad. Returns per-core index arrays and perms."""
    import heapq
    per_core = []
    for c in range(NCORES):
        lo = c * SLOTS
        hi = min((c + 1) * SLOTS, N)
        nodes_c = hi - lo
        m = (dst >= lo) & (dst < hi)
        e_src = src[m].astype(np.int64)
        e_dstl = (dst[m] - lo).astype(np.int64)
        deg = np.bincount(e_dstl, minlength=nodes_c)
        order = np.argsort(-deg, kind="stable")
        # greedy balance: assign node to least-loaded block with a free slot
        heap = [(0, b) for b in range(NBLK)]
        heapq.heapify(heap)
        slots_used = np.zeros(NBLK, np.int64)
        blk_of = np.empty(nodes_c, np.int64)
        slot_of = np.empty(nodes_c, np.int64)
        spill = []
        for v in order:
            while True:
                load, b = heapq.heappop(heap)
                if slots_used[b] < 128:
                    break
                spill.append((load, b))
            blk_of[v] = b
            slot_of[v] = slots_used[b]
            slots_used[b] += 1
            heapq.heappush(heap, (load + int(deg[v]), b))
        eb = blk_of[e_dstl]
        cap = TPB * 128
        cnt = np.bincount(eb, minlength=NBLK)
        assert cnt.max() <= cap, f"block overflow {cnt.max()} > {cap}"
        eorder = np.argsort(eb, kind="stable")
        offs = np.zeros(NBLK + 1, np.int64)
        np.cumsum(cnt, out=offs[1:])
        within = np.arange(len(eb)) - offs[eb[eorder]]
        p_src = np.full((NBLK, cap), SENT, np.int32)
        p_slot = np.full((NBLK, cap), 300.0, np.float32)
        p_src[eb[eorder], within] = e_src[eorder].astype(np.int32)
        p_slot[eb[eorder], within] = slot_of[e_dstl[eorder]].astype(np.float32)
        # node index per slot (-1 for pad slots)
        node_of_slot = np.full(SLOTS, -1, np.int64)
        node_of_slot[blk_of * 128 + slot_of] = np.arange(nodes_c) + lo
        src_arr = p_src.reshape(NBLK, TPB, 128).transpose(2, 0, 1).reshape(128, TILES)
        dslot_arr = p_slot.reshape(NBLK, TPB, 128).transpose(2, 0, 1).reshape(128, TILES)
        drow_arr = p_slot.reshape(1, EDGES_PAD).astype(np.float16)
        per_core.append((src_arr, dslot_arr, drow_arr, node_of_slot))
    return per_core


def kernel(x, src, dst, W, attn_l, attn_r, bias, gamma, beta):
    global LAST_EXEC_NS
    x = np.asarray(x, np.float32)
    src = np.asarray(src, np.int32)
    dst = np.asarray(dst, np.int32)
    W = np.asarray(W, np.float32)
    attn_l = np.asarray(attn_l, np.float32)
    attn_r = np.asarray(attn_r, np.float32)
    gamma = np.asarray(gamma, np.float32)
    beta = np.asarray(beta, np.float32)

    if "l1" not in _cache:
        _cache["l1"] = _build_launch1()
    if "l2" not in _cache:
        _cache["l2"] = _build_launch2()
    nc1, nc2 = _cache["l1"], _cache["l2"]

    per_core = _host_prep(x, src, dst)

    xT_full = np.zeros((128, NTOT), np.float32)
    xT_full[:, :N] = x.T
    xT_hi = xT_full.astype(np.float16)
    xT_lo = (xT_full - xT_hi.astype(np.float32)).astype(np.float16)
    am = np.zeros((HD, 2 * H), np.float32)
    for h in range(H):
        am[h * D:(h + 1) * D, h] = attn_l[h]
        am[h * D:(h + 1) * D, H + h] = attn_r[h]
    iota_r = np.tile(np.arange(128, dtype=np.float16), (128, 1))
    iota_c = np.arange(128, dtype=np.float32).reshape(128, 1)

    in_maps = []
    xTp_list = []
    for c in range(NCORES):
        src_arr, dslot_arr, drow_arr, node_of_slot = per_core[c]
        xTp = np.zeros((128, SLOTS), np.float32)
        real = node_of_slot >= 0
        xTp[:, real] = x[node_of_slot[real]].T
        xTp_list.append(xTp)
        in_maps.append({
            "xTh": xT_hi, "xTl": xT_lo, "xTp": xTp, "W": W, "am": am,
            "iota_r": iota_r, "iota_c": iota_c,
            "srci": src_arr, "dslot": dslot_arr, "drow": drow_arr,
        })

    res1 = run_bass_kernel_spmd(nc1, in_maps, list(range(NCORES)),
                                **_trace_kwargs())
    LAST_EXEC_NS[0] = res1.exec_time_ns or 0

    # host: combine BN stats (2x128 floats per core)
    S1 = np.zeros(128, np.float64)
    S2 = np.zeros(128, np.float64)
    for c in range(NCORES):
        st = res1.results[c]["st_out"]
        S1 += st[:, 0]
        S2 += st[:, 1]
    mu = (S1 / N).astype(np.float32)
    var = (S2 / N - (S1 / N) ** 2).astype(np.float32)
    a = gamma / np.sqrt(var + EPS)
    cc = beta - a * mu
    ac = np.stack([a, cc], axis=1).astype(np.float32)

    in_maps2 = []
    for c in range(NCORES):
        in_maps2.append({
            "h_in": res1.results[c]["h_out"],
            "xTp": xTp_list[c],
            "ac": ac,
        })
    res2 = run_bass_kernel_spmd(nc2, in_maps2, list(range(NCORES)),
                                **_trace_kwargs())
    LAST_EXEC_NS[1] = res2.exec_time_ns or 0

    out = np.zeros((N, IN_DIM), np.float32)
    for c in range(NCORES):
        node_of_slot = per_core[c][3]
        real = node_of_slot >= 0
        ot = res2.results[c]["out_t"]  # [128, SLOTS]
        out[node_of_slot[real]] = ot[:, real].T
    return out


def _trace_kwargs():
    import os
    if os.environ.get("GAT_TRACE", "0") == "1":
        return {"trace": True}
    return {}


# revision 14
# speedup vs baseline: 1.0201x; 1.0201x over previous
"""GAT layer (DGL GATConv + BatchNorm + ELU + residual) on 8 Trainium2 cores.

Strategy (dst-sharded graph parallel):
  - Sort edges by destination; shard destination nodes across 8 cores
    (12544 slots/core = 98 blocks x 128 slots, load-balanced by degree).
  - Each core builds the full node table  [feat | el] = [x@W | x@W@almat]
    (100353 rows x 136 f32; row 100352 is a sentinel with el=-1e30) in its
    own HBM, then gathers 544B rows per edge with indirect DMA.
  - Per 128-edge tile: one-hot S (edge->slot) built on DVE via is_equal;
    er expanded edge-wise with a one-hot matmul; softmax without max
    subtraction (exp range is tiny); messages scaled by w=exp(lrelu(e));
    scatter-reduce into PSUM via S^T @ msg on the PE.
  - BatchNorm batch stats are global: launch 1 returns per-core partial
    sums, the host reduces 2x128 floats, launch 2 applies the affine fold
    a*h+c, ELU and the residual in channel-major layout.
"""
import sys
sys.path.insert(0, "/opt/trn_rl_repo")
import numpy as np

import concourse.bass as bass
import concourse.bacc as bacc
import concourse.mybir as mybir
import concourse.tile as tile
from concourse.bass_utils import run_bass_kernel_spmd

F32 = mybir.dt.float32
F16 = mybir.dt.float16
I32 = mybir.dt.int32

N = 100000
E = 1600000
IN_DIM = 128
H = 8
D = 16
HD = 128
NCORES = 8
NBLK = 98                 # blocks per core
TPB = 17                  # tiles per block
SLOTS = NBLK * 128        # 12544 slots per core
TILES = NBLK * TPB        # 1666 tiles per core
EDGES_PAD = TILES * 128   # padded edge slots per core
NTOT = NCORES * SLOTS     # 100352 padded node count
SENT = NTOT               # sentinel table row
ROW = IN_DIM + H          # 136 psum row (feat | el)
ROW16 = IN_DIM + 2 * H    # 144 fp16 slots per table row (el stored as fp32 pairs)
NEG_SLOPE = 0.2
EPS = 1e-5

LAST_EXEC_NS = [0, 0]

_cache = {}


def _build_launch1():
    nc = bacc.Bacc("TRN2", target_bir_lowering=False, debug=False,
                   num_devices=NCORES)
    xTh = nc.dram_tensor("xTh", [128, NTOT], F16, kind="ExternalInput")
    xTl = nc.dram_tensor("xTl", [128, NTOT], F16, kind="ExternalInput")
    xTp = nc.dram_tensor("xTp", [128, SLOTS], F32, kind="ExternalInput")
    Wd = nc.dram_tensor("W", [IN_DIM, HD], F32, kind="ExternalInput")
    amd = nc.dram_tensor("am", [HD, 2 * H], F32, kind="ExternalInput")
    iota_r = nc.dram_tensor("iota_r", [128, 128], F16, kind="ExternalInput")
    iota_c = nc.dram_tensor("iota_c", [128, 1], F32, kind="ExternalInput")
    srcd = nc.dram_tensor("srci", [128, TILES], I32, kind="ExternalInput")
    dslotd = nc.dram_tensor("dslot", [128, TILES], F32, kind="ExternalInput")
    drowd = nc.dram_tensor("drow", [1, EDGES_PAD], F16, kind="ExternalInput")

    h_out = nc.dram_tensor("h_out", [SLOTS, HD], F32, kind="ExternalOutput")
    st_out = nc.dram_tensor("st_out", [128, 2], F32, kind="ExternalOutput")
    table = nc.dram_tensor("table", [NTOT + 1, ROW16], F16)

    NT_A = NTOT // 128  # 784 node tiles for table build

    with tile.TileContext(nc) as tc:
        with (
            tc.tile_pool(name="const", bufs=1) as constp,
            tc.tile_pool(name="pa_sb", bufs=4) as pa_sb,
            tc.tile_pool(name="ers", bufs=1) as ersp,
            tc.tile_pool(name="g4p", bufs=12) as g4p,
            tc.tile_pool(name="sp", bufs=8) as sp,
            tc.tile_pool(name="st4p", bufs=5) as st4p,
            tc.tile_pool(name="wp", bufs=6) as wp,
            tc.tile_pool(name="drp", bufs=3) as drp,
            tc.tile_pool(name="fin", bufs=3) as finp,
        ):
            # ---- constants ----
            iota_row = constp.tile([128, 128], F16)
            nc.sync.dma_start(out=iota_row[:], in_=iota_r[:])
            iota_col = constp.tile([128, 1], F32)
            nc.sync.dma_start(out=iota_col[:], in_=iota_c[:])
            ones_row = constp.tile([1, 128], F16)
            nc.vector.memset(ones_row[:], 1.0)
            ones_col = constp.tile([128, 1], F32)
            nc.vector.memset(ones_col[:], 1.0)
            ones_col16 = constp.tile([128, 1], F16)
            nc.vector.memset(ones_col16[:], 1.0)

            pa_scope = tc.tile_pool(name="pa_ps", bufs=4, space="PSUM")
            pa_ps = pa_scope.__enter__()
            # ---- Wfull = [W | W@almat | W@armat]  [128, 144] ----
            W_sb = constp.tile([128, HD], F32)
            nc.sync.dma_start(out=W_sb[:], in_=Wd[:])
            am_sb = constp.tile([128, 2 * H], F32)
            nc.sync.dma_start(out=am_sb[:], in_=amd[:])
            ident = constp.tile([128, 128], F32)
            from concourse.masks import make_identity
            make_identity(nc, ident[:])
            wt_ps = pa_ps.tile([128, 128], F32, tag="pa")
            nc.tensor.transpose(out=wt_ps[:], in_=W_sb[:], identity=ident[:])
            WT_sb = constp.tile([128, 128], F32)
            nc.vector.tensor_copy(out=WT_sb[:], in_=wt_ps[:])
            Wfull = constp.tile([128, IN_DIM + 2 * H], F32)
            nc.vector.tensor_copy(out=Wfull[:, 0:HD], in_=W_sb[:])
            wlr_ps = pa_ps.tile([128, 2 * H], F32, tag="pa")
            nc.tensor.matmul(out=wlr_ps[:], lhsT=WT_sb[:], rhs=am_sb[:],
                             start=True, stop=True)
            nc.vector.tensor_copy(out=Wfull[:, HD:HD + 2 * H], in_=wlr_ps[:])
            Wh = constp.tile([128, IN_DIM + 2 * H], F16)
            nc.vector.tensor_copy(out=Wh[:], in_=Wfull[:])
            Wh32 = constp.tile([128, IN_DIM + 2 * H], F32)
            nc.vector.tensor_copy(out=Wh32[:], in_=Wh[:])
            Wl = constp.tile([128, IN_DIM + 2 * H], F16)
            nc.vector.tensor_tensor(out=Wl[:], in0=Wfull[:], in1=Wh32[:],
                                    op=mybir.AluOpType.subtract)

            # ---- sentinel row ----
            sent_sb = constp.tile([1, ROW16], F16)
            nc.vector.memset(sent_sb[:], 0.0)
            nc.vector.memset(sent_sb[:, IN_DIM:ROW16].bitcast(F32), -1e30)
            nc.sync.dma_start(out=table[SENT:SENT + 1, :], in_=sent_sb[:])

            # ---- phase A: full node table (groups of 4 tiles) ----
            for t4 in range(NT_A // 4):
                x4h = pa_sb.tile([128, 512], F16, tag="xth")
                nc.scalar.dma_start(out=x4h[:], in_=xTh[:, t4 * 512:(t4 + 1) * 512])
                x4l = pa_sb.tile([128, 512], F16, tag="xtl")
                nc.scalar.dma_start(out=x4l[:], in_=xTl[:, t4 * 512:(t4 + 1) * 512])
                row4 = pa_sb.tile([128, 4 * ROW16], F16, tag="row4")
                for k in range(4):
                    ps = pa_ps.tile([128, ROW], F32, tag="pa")
                    nc.tensor.matmul(out=ps[:], lhsT=x4h[:, k * 128:(k + 1) * 128],
                                     rhs=Wh[:, 0:ROW], start=True, stop=False)
                    nc.tensor.matmul(out=ps[:], lhsT=x4h[:, k * 128:(k + 1) * 128],
                                     rhs=Wl[:, 0:ROW], start=False, stop=False)
                    nc.tensor.matmul(out=ps[:], lhsT=x4l[:, k * 128:(k + 1) * 128],
                                     rhs=Wh[:, 0:ROW], start=False, stop=True)
                    o = k * ROW16
                    if k % 2 == 0:
                        nc.vector.tensor_copy(out=row4[:, o:o + IN_DIM],
                                              in_=ps[:, 0:IN_DIM])
                    else:
                        nc.scalar.activation(row4[:, o:o + IN_DIM],
                                             ps[:, 0:IN_DIM],
                                             mybir.ActivationFunctionType.Copy)
                    nc.vector.tensor_copy(
                        out=row4[:, o + IN_DIM:o + ROW16].bitcast(F32),
                        in_=ps[:, IN_DIM:ROW])
                nc.sync.dma_start(
                    out=table[t4 * 512:(t4 + 1) * 512, :].rearrange(
                        "(f p) c -> p f c", f=4),
                    in_=row4[:].rearrange("p (f c) -> p f c", c=ROW16))

            # ---- er for own slots: hi/lo fp16 pairs [128, 98*16] ----
            er_sb = ersp.tile([128, NBLK * 2 * H], F16)
            for b in range(NBLK):
                xp_sb = pa_sb.tile([128, 128], F32, tag="xp")
                nc.scalar.dma_start(out=xp_sb[:], in_=xTp[:, b * 128:(b + 1) * 128])
                ps = pa_ps.tile([128, H], F32, tag="pa")
                nc.tensor.matmul(out=ps[:], lhsT=xp_sb[:],
                                 rhs=Wfull[:, ROW:ROW + H], start=True, stop=True)
                o = b * 2 * H
                nc.vector.tensor_copy(out=er_sb[:, o:o + H], in_=ps[:])
                hi32 = finp.tile([128, H], F32, tag="hi32")
                nc.vector.tensor_copy(out=hi32[:], in_=er_sb[:, o:o + H])
                nc.vector.tensor_tensor(out=er_sb[:, o + H:o + 2 * H],
                                        in0=ps[:], in1=hi32[:],
                                        op=mybir.AluOpType.subtract)

            pa_scope.__exit__(None, None, None)
            blk_scope = tc.tile_pool(name="blk_ps", bufs=2, space="PSUM")
            blk_ps = blk_scope.__enter__()
            erp_scope = tc.tile_pool(name="er_ps", bufs=2, space="PSUM")
            er_ps = erp_scope.__enter__()
            dt_scope = tc.tile_pool(name="dt_ps", bufs=2, space="PSUM")
            dt_ps = dt_scope.__enter__()
            st_scope = tc.tile_pool(name="stat_ps", bufs=1, space="PSUM")
            stat_ps = st_scope.__enter__()
            # ---- index preloads ----
            src_sb = constp.tile([128, TILES], I32)
            nc.sync.dma_start(out=src_sb[:], in_=srcd[:])
            dslot_sb = constp.tile([128, TILES], F32)
            nc.sync.dma_start(out=dslot_sb[:], in_=dslotd[:])

            # ---- stats accumulators (persist across blocks) ----
            s1_ps = stat_ps.tile([128, 1], F32)
            s2_ps = stat_ps.tile([128, 1], F32)

            GPB = TPB // 4 + (1 if TPB % 4 else 0)  # groups per block (of <=4 tiles)

            # ---- phase B ----
            for b in range(NBLK):
                dr = drp.tile([1, TPB * 128], F16, tag="dr")
                nc.sync.dma_start(out=dr[:],
                                  in_=drowd[:, b * TPB * 128:(b + 1) * TPB * 128])
                psb = blk_ps.tile([128, ROW], F32, tag="blk")
                for g in range(GPB):
                    t0 = g * 4
                    nt = min(4, TPB - t0)
                    ne = nt * 128
                    # replicate dst slots across partitions, build ST
                    dtp = dt_ps.tile([128, 512], F32, tag="dt")
                    nc.tensor.matmul(out=dtp[:, :ne], lhsT=ones_row[:],
                                     rhs=dr[:, t0 * 128:t0 * 128 + ne],
                                     start=True, stop=True)
                    st4 = st4p.tile([128, 512], F16, tag="st4")
                    nc.vector.tensor_scalar(out=st4[:, :ne], in0=dtp[:, :ne],
                                            scalar1=iota_col[:],
                                            scalar2=None,
                                            op0=mybir.AluOpType.is_equal)
                    # gather 4 tiles worth of table rows
                    g4 = g4p.tile([128, 4 * ROW16], F16, tag="g4")
                    for k in range(nt):
                        col = b * TPB + t0 + k
                        nc.gpsimd.indirect_dma_start(
                            out=g4[:, k * ROW16:(k + 1) * ROW16],
                            out_offset=None,
                            in_=table[:],
                            in_offset=bass.IndirectOffsetOnAxis(
                                ap=src_sb[:, col:col + 1], axis=0),
                        )
                    # er per edge via one-hot matmul
                    erp = er_ps.tile([128, 4 * 2 * H], F32, tag="erp")
                    for k in range(nt):
                        nc.tensor.matmul(
                            out=erp[:, k * 2 * H:(k + 1) * 2 * H],
                            lhsT=st4[:, k * 128:(k + 1) * 128],
                            rhs=er_sb[:, b * 2 * H:(b + 1) * 2 * H],
                            start=True, stop=True)
                    # e = el + er ; w = exp(lrelu(e))
                    wsb = wp.tile([128, 4 * H], F32, tag="w")
                    el_view = (g4[:].rearrange("p (t c) -> p t c", c=ROW16)
                               [:, 0:nt, IN_DIM:ROW16].bitcast(F32))
                    erp_v = erp[:, :nt * 2 * H].rearrange("p (t u) -> p t u", u=2 * H)
                    w_v = wsb[:, :nt * H].rearrange("p (t h) -> p t h", h=H)
                    nc.vector.tensor_tensor(
                        out=w_v, in0=el_view, in1=erp_v[:, :, 0:H],
                        op=mybir.AluOpType.add)
                    nc.vector.tensor_tensor(
                        out=w_v, in0=w_v, in1=erp_v[:, :, H:2 * H],
                        op=mybir.AluOpType.add)
                    w5 = wp.tile([128, 4 * H], F32, tag="w5")
                    nc.vector.tensor_scalar(out=w5[:, :nt * H],
                                            in0=wsb[:, :nt * H],
                                            scalar1=NEG_SLOPE, scalar2=None,
                                            op0=mybir.AluOpType.mult)
                    nc.vector.tensor_tensor(out=wsb[:, :nt * H],
                                            in0=wsb[:, :nt * H],
                                            in1=w5[:, :nt * H],
                                            op=mybir.AluOpType.max)
                    nc.scalar.activation(wsb[:, :nt * H], wsb[:, :nt * H],
                                         mybir.ActivationFunctionType.Exp)
                    # w into fp16 slots 128:136; scale messages
                    g4r = g4[:].rearrange("p (t c) -> p t c", c=ROW16)
                    w16_view = g4r[:, 0:nt, IN_DIM:IN_DIM + H]
                    nc.scalar.activation(
                        w16_view,
                        wsb[:, :nt * H].rearrange("p (t h) -> p t h", h=H),
                        mybir.ActivationFunctionType.Copy)
                    feat_view = g4r[:, 0:nt, 0:IN_DIM]
                    w_b = (w16_view
                           .rearrange("p t (h one) -> p t h one", h=H, one=1)
                           .to_broadcast([128, nt, H, D]))
                    nc.vector.tensor_tensor(
                        out=feat_view.rearrange("p t (h d) -> p t h d", d=D),
                        in0=feat_view.rearrange("p t (h d) -> p t h d", d=D),
                        in1=w_b,
                        op=mybir.AluOpType.mult)
                    # per-tile one-hot S + scatter matmul
                    for k in range(nt):
                        col = b * TPB + t0 + k
                        s_sb = sp.tile([128, 128], F16, tag="s")
                        nc.vector.tensor_scalar(
                            out=s_sb[:], in0=iota_row[:],
                            scalar1=dslot_sb[:, col:col + 1],
                            scalar2=None,
                            op0=mybir.AluOpType.is_equal)
                        ti = t0 + k
                        nc.tensor.matmul(out=psb[:],
                                         lhsT=s_sb[:],
                                         rhs=g4[:, k * ROW16:k * ROW16 + ROW],
                                         start=(ti == 0), stop=(ti == TPB - 1))
                # ---- block finalize ----
                ssum = finp.tile([128, H], F32, tag="ssum")
                nc.vector.tensor_scalar(out=ssum[:], in0=psb[:, IN_DIM:ROW],
                                        scalar1=1e-30, scalar2=None,
                                        op0=mybir.AluOpType.add)
                rec = finp.tile([128, H], F32, tag="rec")
                nc.vector.reciprocal(out=rec[:], in_=ssum[:])
                h_sb = finp.tile([128, HD], F32, tag="h")
                rec_b = (rec[:].rearrange("p (h one) -> p h one", h=H, one=1)
                         .to_broadcast([128, H, D]))
                nc.vector.tensor_tensor(
                    out=h_sb[:].rearrange("p (h d) -> p h d", d=D),
                    in0=psb[:, 0:IN_DIM].rearrange("p (h d) -> p h d", d=D),
                    in1=rec_b, op=mybir.AluOpType.mult)
                h16 = finp.tile([128, HD], F16, tag="h16")
                nc.vector.tensor_copy(out=h16[:], in_=h_sb[:])
                sq_sb = finp.tile([128, HD], F16, tag="sq")
                nc.scalar.activation(sq_sb[:], h_sb[:],
                                     mybir.ActivationFunctionType.Square)
                nc.tensor.matmul(out=s1_ps[:], lhsT=h16[:], rhs=ones_col16[:],
                                 start=(b == 0), stop=(b == NBLK - 1))
                nc.tensor.matmul(out=s2_ps[:], lhsT=sq_sb[:], rhs=ones_col16[:],
                                 start=(b == 0), stop=(b == NBLK - 1))
                nc.sync.dma_start(out=h_out[b * 128:(b + 1) * 128, :], in_=h_sb[:])

            stat_sb = constp.tile([128, 2], F32)
            nc.vector.tensor_copy(out=stat_sb[:, 0:1], in_=s1_ps[:])
            nc.vector.tensor_copy(out=stat_sb[:, 1:2], in_=s2_ps[:])
            nc.sync.dma_start(out=st_out[:], in_=stat_sb[:])
            st_scope.__exit__(None, None, None)
            dt_scope.__exit__(None, None, None)
            erp_scope.__exit__(None, None, None)
            blk_scope.__exit__(None, None, None)

    nc.compile()
    return nc


def _build_launch2():
    nc = bacc.Bacc("TRN2", target_bir_lowering=False, debug=False,
                   num_devices=NCORES)
    h_in = nc.dram_tensor("h_in", [SLOTS, HD], F32, kind="ExternalInput")
    xTp = nc.dram_tensor("xTp", [128, SLOTS], F32, kind="ExternalInput")
    ac = nc.dram_tensor("ac", [128, 2], F32, kind="ExternalInput")
    out_t = nc.dram_tensor("out_t", [128, SLOTS], F32, kind="ExternalOutput")

    CH = 512
    NCH = SLOTS // CH  # 24.5 -> handle 24 full + 1 tail of 256
    chunks = [(i * CH, CH) for i in range(NCH)]
    if SLOTS % CH:
        chunks.append((NCH * CH, SLOTS % CH))

    with tile.TileContext(nc) as tc:
        with (
            tc.tile_pool(name="const", bufs=1) as constp,
            tc.tile_pool(name="ld", bufs=4) as ldp,
            tc.tile_pool(name="ps", bufs=3, space="PSUM") as psp,
            tc.tile_pool(name="wk", bufs=3) as wkp,
        ):
            from concourse.masks import make_identity
            ident = constp.tile([128, 128], F32)
            make_identity(nc, ident[:])
            ac_sb = constp.tile([128, 2], F32)
            nc.sync.dma_start(out=ac_sb[:], in_=ac[:])

            for (o, w) in chunks:
                nk = w // 128
                hp = psp.tile([128, CH], F32, tag="hp")
                for k in range(nk):
                    hl = ldp.tile([128, 128], F32, tag="hl")
                    nc.sync.dma_start(
                        out=hl[:], in_=h_in[o + k * 128:o + (k + 1) * 128, :])
                    nc.tensor.transpose(out=hp[:, k * 128:(k + 1) * 128],
                                        in_=hl[:], identity=ident[:])
                h2 = wkp.tile([128, CH], F32, tag="h2")
                nc.vector.tensor_scalar(out=h2[:, :w], in0=hp[:, :w],
                                        scalar1=ac_sb[:, 0:1],
                                        scalar2=ac_sb[:, 1:2],
                                        op0=mybir.AluOpType.mult,
                                        op1=mybir.AluOpType.add)
                m = wkp.tile([128, CH], F32, tag="m")
                nc.vector.tensor_scalar(out=m[:, :w], in0=h2[:, :w],
                                        scalar1=0.0, scalar2=None,
                                        op0=mybir.AluOpType.min)
                nc.scalar.activation(m[:, :w], m[:, :w],
                                     mybir.ActivationFunctionType.Exp)
                nc.vector.tensor_scalar(out=m[:, :w], in0=m[:, :w],
                                        scalar1=-1.0, scalar2=None,
                                        op0=mybir.AluOpType.add)
                # elu = max(h2, exp(min(h2,0))-1)
                nc.vector.tensor_tensor(out=h2[:, :w], in0=h2[:, :w],
                                        in1=m[:, :w],
                                        op=mybir.AluOpType.max)
                xt = ldp.tile([128, CH], F32, tag="xt")
                nc.sync.dma_start(out=xt[:, :w], in_=xTp[:, o:o + w])
                nc.vector.tensor_tensor(out=h2[:, :w], in0=h2[:, :w],
                                        in1=xt[:, :w], op=mybir.AluOpType.add)
                nc.sync.dma_start(out=out_t[:, o:o + w], in_=h2[:, :w])

    nc.compile()
    return nc


def _host_prep(x, src, dst):
    """Shard + balance + pad. Returns per-core index arrays and perms."""
    import heapq
    per_core = []
    for c in range(NCORES):
        lo = c * SLOTS
        hi = min((c + 1) * SLOTS, N)
        nodes_c = hi - lo
        m = (dst >= lo) & (dst < hi)
        e_src = src[m].astype(np.int64)
        e_dstl = (dst[m] - lo).astype(np.int64)
        deg = np.bincount(e_dstl, minlength=nodes_c)
        order = np.argsort(-deg, kind="stable")
        # greedy balance: assign node to least-loaded block with a free slot
        heap = [(0, b) for b in range(NBLK)]
        heapq.heapify(heap)
        slots_used = np.zeros(NBLK, np.int64)
        blk_of = np.empty(nodes_c, np.int64)
        slot_of = np.empty(nodes_c, np.int64)
        spill = []
        for v in order:
            while True:
                load, b = heapq.heappop(heap)
                if slots_used[b] < 128:
                    break
                spill.append((load, b))
            blk_of[v] = b
            slot_of[v] = slots_used[b]
            slots_used[b] += 1
            heapq.heappush(heap, (load + int(deg[v]), b))
        eb = blk_of[e_dstl]
        cap = TPB * 128
        cnt = np.bincount(eb, minlength=NBLK)
        assert cnt.max() <= cap, f"block overflow {cnt.max()} > {cap}"
        eorder = np.argsort(eb, kind="stable")
        offs = np.zeros(NBLK + 1, np.int64)
        np.cumsum(cnt, out=offs[1:])
        within = np.arange(len(eb)) - offs[eb[eorder]]
        p_src = np.full((NBLK, cap), SENT, np.int32)
        p_slot = np.full((NBLK, cap), 300.0, np.float32)
        p_src[eb[eorder], within] = e_src[eorder].astype(np.int32)
        p_slot[eb[eorder], within] = slot_of[e_dstl[eorder]].astype(np.float32)
        # node index per slot (-1 for pad slots)
        node_of_slot = np.full(SLOTS, -1, np.int64)
        node_of_slot[blk_of * 128 + slot_of] = np.arange(nodes_c) + lo
        src_arr = p_src.reshape(NBLK, TPB, 128).transpose(2, 0, 1).reshape(128, TILES)
        dslot_arr = p_slot.reshape(NBLK, TPB, 128).transpose(2, 0, 1).reshape(128, TILES)
        drow_arr = p_slot.reshape(1, EDGES_PAD).astype(np.float16)
        per_core.append((src_arr, dslot_arr, drow_arr, node_of_slot))
    return per_core


def kernel(x, src, dst, W, attn_l, attn_r, bias, gamma, beta):
    global LAST_EXEC_NS
    x = np.asarray(x, np.float32)
    src = np.asarray(src, np.int32)
    dst = np.asarray(dst, np.int32)
    W = np.asarray(W, np.float32)
    attn_l = np.asarray(attn_l, np.float32)
    attn_r = np.asarray(attn_r, np.float32)
    gamma = np.asarray(gamma, np.float32)
    beta = np.asarray(beta, np.float32)

    if "l1" not in _cache:
        _cache["l1"] = _build_launch1()
    if "l2" not in _cache:
        _cache["l2"] = _build_launch2()
    nc1, nc2 = _cache["l1"], _cache["l2"]

    per_core = _host_prep(x, src, dst)

    xT_full = np.zeros((128, NTOT), np.float32)
    xT_full[:, :N] = x.T
    xT_hi = xT_full.astype(np.float16)
    xT_lo = (xT_full - xT_hi.astype(np.float32)).astype(np.float16)
    am = np.zeros((HD, 2 * H), np.float32)
    for h in range(H):
        am[h * D:(h + 1) * D, h] = attn_l[h]
        am[h * D:(h + 1) * D, H + h] = attn_r[h]
    iota_r = np.tile(np.arange(128, dtype=np.float16), (128, 1))
    iota_c = np.arange(128, dtype=np.float32).reshape(128, 1)

    in_maps = []
    xTp_list = []
    for c in range(NCORES):
        src_arr, dslot_arr, drow_arr, node_of_slot = per_core[c]
        xTp = np.zeros((128, SLOTS), np.float32)
        real = node_of_slot >= 0
        xTp[:, real] = x[node_of_slot[real]].T
        xTp_list.append(xTp)
        in_maps.append({
            "xTh": xT_hi, "xTl": xT_lo, "xTp": xTp, "W": W, "am": am,
            "iota_r": iota_r, "iota_c": iota_c,
            "srci": src_arr, "dslot": dslot_arr, "drow": drow_arr,
        })

    res1 = run_bass_kernel_spmd(nc1, in_maps, list(range(NCORES)),
                                **_trace_kwargs())
    LAST_EXEC_NS[0] = res1.exec_time_ns or 0

    # host: combine BN stats (2x128 floats per core)
    S1 = np.zeros(128, np.float64)
    S2 = np.zeros(128, np.float64)
    for c in range(NCORES):
        st = res1.results[c]["st_out"]
        S1 += st[:, 0]
        S2 += st[:, 1]
    mu = (S1 / N).astype(np.float32)
    var = (S2 / N - (S1 / N) ** 2).astype(np.float32)
    a = gamma / np.sqrt(var + EPS)
    cc = beta - a * mu
    ac = np.stack([a, cc], axis=1).astype(np.float32)

    in_maps2 = []
    for c in range(NCORES):
        in_maps2.append({
            "h_in": res1.results[c]["h_out"],
            "xTp": xTp_list[c],
            "ac": ac,
        })
    res2 = run_bass_kernel_spmd(nc2, in_maps2, list(range(NCORES)),
                                **_trace_kwargs())
    LAST_EXEC_NS[1] = res2.exec_time_ns or 0

    out = np.zeros((N, IN_DIM), np.float32)
    for c in range(NCORES):
        node_of_slot = per_core[c][3]
        real = node_of_slot >= 0
        ot = res2.results[c]["out_t"]  # [128, SLOTS]
        out[node_of_slot[real]] = ot[:, real].T
    return out


def _trace_kwargs():
    import os
    if os.environ.get("GAT_TRACE", "0") == "1":
        return {"trace": True}
    return {}


# revision 15
# speedup vs baseline: 1.0372x; 1.0168x over previous
"""GAT layer (DGL GATConv + BatchNorm + ELU + residual) on 8 Trainium2 cores.

Strategy (dst-sharded graph parallel):
  - Sort edges by destination; shard destination nodes across 8 cores
    (12544 slots/core = 98 blocks x 128 slots, load-balanced by degree).
  - Each core builds the full node table  [feat | el] = [x@W | x@W@almat]
    (100353 rows x 136 f32; row 100352 is a sentinel with el=-1e30) in its
    own HBM, then gathers 544B rows per edge with indirect DMA.
  - Per 128-edge tile: one-hot S (edge->slot) built on DVE via is_equal;
    er expanded edge-wise with a one-hot matmul; softmax without max
    subtraction (exp range is tiny); messages scaled by w=exp(lrelu(e));
    scatter-reduce into PSUM via S^T @ msg on the PE.
  - BatchNorm batch stats are global: launch 1 returns per-core partial
    sums, the host reduces 2x128 floats, launch 2 applies the affine fold
    a*h+c, ELU and the residual in channel-major layout.
"""
import sys
sys.path.insert(0, "/opt/trn_rl_repo")
import numpy as np

import concourse.bass as bass
import concourse.bacc as bacc
import concourse.mybir as mybir
import concourse.tile as tile
from concourse.bass_utils import run_bass_kernel_spmd

F32 = mybir.dt.float32
F16 = mybir.dt.float16
I32 = mybir.dt.int32

N = 100000
E = 1600000
IN_DIM = 128
H = 8
D = 16
HD = 128
NCORES = 8
NBLK = 98                 # blocks per core
TPB = 17                  # tiles per block
SLOTS = NBLK * 128        # 12544 slots per core
TILES = NBLK * TPB        # 1666 tiles per core
EDGES_PAD = TILES * 128   # padded edge slots per core
NTOT = NCORES * SLOTS     # 100352 padded node count
SENT = NTOT               # sentinel table row
ROW = IN_DIM + H          # 136 psum row (feat | el)
ROW16 = IN_DIM + 2 * H    # 144 fp16 slots per table row (el stored as fp32 pairs)
NEG_SLOPE = 0.2
EPS = 1e-5

LAST_EXEC_NS = [0, 0]

_cache = {}


def _build_launch1():
    nc = bacc.Bacc("TRN2", target_bir_lowering=False, debug=False,
                   num_devices=NCORES)
    xTh = nc.dram_tensor("xTh", [128, NTOT], F16, kind="ExternalInput")
    xTl = nc.dram_tensor("xTl", [128, NTOT], F16, kind="ExternalInput")
    xTp = nc.dram_tensor("xTp", [128, SLOTS], F32, kind="ExternalInput")
    Wd = nc.dram_tensor("W", [IN_DIM, HD], F32, kind="ExternalInput")
    amd = nc.dram_tensor("am", [HD, 2 * H], F32, kind="ExternalInput")
    iota_r = nc.dram_tensor("iota_r", [128, 128], F16, kind="ExternalInput")
    iota_c = nc.dram_tensor("iota_c", [128, 1], F32, kind="ExternalInput")
    srcd = nc.dram_tensor("srci", [128, TILES], I32, kind="ExternalInput")
    dslotd = nc.dram_tensor("dslot", [128, TILES], F32, kind="ExternalInput")
    drowd = nc.dram_tensor("drow", [1, EDGES_PAD], F16, kind="ExternalInput")

    h_out = nc.dram_tensor("h_out", [SLOTS, HD], F32, kind="ExternalOutput")
    st_out = nc.dram_tensor("st_out", [128, 2], F32, kind="ExternalOutput")
    table = nc.dram_tensor("table", [NTOT + 1, ROW16], F16)

    NT_A = NTOT // 128  # 784 node tiles for table build

    with tile.TileContext(nc) as tc:
        with (
            tc.tile_pool(name="const", bufs=1) as constp,
            tc.tile_pool(name="pa_sb", bufs=6) as pa_sb,
            tc.tile_pool(name="ers", bufs=1) as ersp,
            tc.tile_pool(name="g4p", bufs=12) as g4p,
            tc.tile_pool(name="sp", bufs=8) as sp,
            tc.tile_pool(name="st4p", bufs=5) as st4p,
            tc.tile_pool(name="wp", bufs=6) as wp,
            tc.tile_pool(name="drp", bufs=3) as drp,
            tc.tile_pool(name="fin", bufs=3) as finp,
        ):
            # ---- constants ----
            iota_row = constp.tile([128, 128], F16)
            nc.sync.dma_start(out=iota_row[:], in_=iota_r[:])
            iota_col = constp.tile([128, 1], F32)
            nc.sync.dma_start(out=iota_col[:], in_=iota_c[:])
            ones_row = constp.tile([1, 128], F16)
            nc.vector.memset(ones_row[:], 1.0)
            ones_col = constp.tile([128, 1], F32)
            nc.vector.memset(ones_col[:], 1.0)
            ones_col16 = constp.tile([128, 1], F16)
            nc.vector.memset(ones_col16[:], 1.0)

            pa_scope = tc.tile_pool(name="pa_ps", bufs=7, space="PSUM")
            pa_ps = pa_scope.__enter__()
            # ---- Wfull = [W | W@almat | W@armat]  [128, 144] ----
            W_sb = constp.tile([128, HD], F32)
            nc.sync.dma_start(out=W_sb[:], in_=Wd[:])
            am_sb = constp.tile([128, 2 * H], F32)
            nc.sync.dma_start(out=am_sb[:], in_=amd[:])
            ident = constp.tile([128, 128], F32)
            from concourse.masks import make_identity
            make_identity(nc, ident[:])
            wt_ps = pa_ps.tile([128, 128], F32, tag="pa")
            nc.tensor.transpose(out=wt_ps[:], in_=W_sb[:], identity=ident[:])
            WT_sb = constp.tile([128, 128], F32)
            nc.vector.tensor_copy(out=WT_sb[:], in_=wt_ps[:])
            Wfull = constp.tile([128, IN_DIM + 2 * H], F32)
            nc.vector.tensor_copy(out=Wfull[:, 0:HD], in_=W_sb[:])
            wlr_ps = pa_ps.tile([128, 2 * H], F32, tag="pa")
            nc.tensor.matmul(out=wlr_ps[:], lhsT=WT_sb[:], rhs=am_sb[:],
                             start=True, stop=True)
            nc.vector.tensor_copy(out=Wfull[:, HD:HD + 2 * H], in_=wlr_ps[:])
            Wh = constp.tile([128, IN_DIM + 2 * H], F16)
            nc.vector.tensor_copy(out=Wh[:], in_=Wfull[:])
            Wh32 = constp.tile([128, IN_DIM + 2 * H], F32)
            nc.vector.tensor_copy(out=Wh32[:], in_=Wh[:])
            Wl = constp.tile([128, IN_DIM + 2 * H], F16)
            nc.vector.tensor_tensor(out=Wl[:], in0=Wfull[:], in1=Wh32[:],
                                    op=mybir.AluOpType.subtract)

            # ---- sentinel row ----
            sent_sb = constp.tile([1, ROW16], F16)
            nc.vector.memset(sent_sb[:], 0.0)
            nc.vector.memset(sent_sb[:, IN_DIM:ROW16].bitcast(F32), -1e30)
            nc.sync.dma_start(out=table[SENT:SENT + 1, :], in_=sent_sb[:])

            # ---- phase A: full node table (groups of 4 tiles) ----
            for t4 in range(NT_A // 4):
                x4h = pa_sb.tile([128, 512], F16, tag="xth")
                nc.scalar.dma_start(out=x4h[:], in_=xTh[:, t4 * 512:(t4 + 1) * 512])
                x4l = pa_sb.tile([128, 512], F16, tag="xtl")
                nc.sync.dma_start(out=x4l[:], in_=xTl[:, t4 * 512:(t4 + 1) * 512])
                row4 = pa_sb.tile([128, 4 * ROW16], F16, tag="row4")
                for k in range(4):
                    ps = pa_ps.tile([128, ROW], F32, tag="pa")
                    nc.tensor.matmul(out=ps[:], lhsT=x4h[:, k * 128:(k + 1) * 128],
                                     rhs=Wh[:, 0:ROW], start=True, stop=False)
                    nc.tensor.matmul(out=ps[:], lhsT=x4h[:, k * 128:(k + 1) * 128],
                                     rhs=Wl[:, 0:ROW], start=False, stop=False)
                    nc.tensor.matmul(out=ps[:], lhsT=x4l[:, k * 128:(k + 1) * 128],
                                     rhs=Wh[:, 0:ROW], start=False, stop=True)
                    o = k * ROW16
                    if k % 2 == 0:
                        nc.vector.tensor_copy(out=row4[:, o:o + IN_DIM],
                                              in_=ps[:, 0:IN_DIM])
                    else:
                        nc.scalar.activation(row4[:, o:o + IN_DIM],
                                             ps[:, 0:IN_DIM],
                                             mybir.ActivationFunctionType.Copy)
                    nc.vector.tensor_copy(
                        out=row4[:, o + IN_DIM:o + ROW16].bitcast(F32),
                        in_=ps[:, IN_DIM:ROW])
                nc.sync.dma_start(
                    out=table[t4 * 512:(t4 + 1) * 512, :].rearrange(
                        "(f p) c -> p f c", f=4),
                    in_=row4[:].rearrange("p (f c) -> p f c", c=ROW16))

            # ---- er for own slots: hi/lo fp16 pairs [128, 98*16] ----
            er_sb = ersp.tile([128, NBLK * 2 * H], F16)
            for b in range(NBLK):
                xp_sb = pa_sb.tile([128, 128], F32, tag="xp")
                nc.scalar.dma_start(out=xp_sb[:], in_=xTp[:, b * 128:(b + 1) * 128])
                ps = pa_ps.tile([128, H], F32, tag="pa")
                nc.tensor.matmul(out=ps[:], lhsT=xp_sb[:],
                                 rhs=Wfull[:, ROW:ROW + H], start=True, stop=True)
                o = b * 2 * H
                nc.vector.tensor_copy(out=er_sb[:, o:o + H], in_=ps[:])
                hi32 = finp.tile([128, H], F32, tag="hi32")
                nc.vector.tensor_copy(out=hi32[:], in_=er_sb[:, o:o + H])
                nc.vector.tensor_tensor(out=er_sb[:, o + H:o + 2 * H],
                                        in0=ps[:], in1=hi32[:],
                                        op=mybir.AluOpType.subtract)

            pa_scope.__exit__(None, None, None)
            blk_scope = tc.tile_pool(name="blk_ps", bufs=2, space="PSUM")
            blk_ps = blk_scope.__enter__()
            erp_scope = tc.tile_pool(name="er_ps", bufs=2, space="PSUM")
            er_ps = erp_scope.__enter__()
            dt_scope = tc.tile_pool(name="dt_ps", bufs=2, space="PSUM")
            dt_ps = dt_scope.__enter__()
            st_scope = tc.tile_pool(name="stat_ps", bufs=1, space="PSUM")
            stat_ps = st_scope.__enter__()
            # ---- index preloads ----
            src_sb = constp.tile([128, TILES], I32)
            nc.sync.dma_start(out=src_sb[:], in_=srcd[:])
            dslot_sb = constp.tile([128, TILES], F32)
            nc.sync.dma_start(out=dslot_sb[:], in_=dslotd[:])

            # ---- stats accumulators (persist across blocks) ----
            s1_ps = stat_ps.tile([128, 1], F32)
            s2_ps = stat_ps.tile([128, 1], F32)

            GPB = TPB // 4 + (1 if TPB % 4 else 0)  # groups per block (of <=4 tiles)

            # ---- phase B ----
            for b in range(NBLK):
                dr = drp.tile([1, TPB * 128], F16, tag="dr")
                nc.sync.dma_start(out=dr[:],
                                  in_=drowd[:, b * TPB * 128:(b + 1) * TPB * 128])
                psb = blk_ps.tile([128, ROW], F32, tag="blk")
                for g in range(GPB):
                    t0 = g * 4
                    nt = min(4, TPB - t0)
                    ne = nt * 128
                    # replicate dst slots across partitions, build ST
                    dtp = dt_ps.tile([128, 512], F32, tag="dt")
                    nc.tensor.matmul(out=dtp[:, :ne], lhsT=ones_row[:],
                                     rhs=dr[:, t0 * 128:t0 * 128 + ne],
                                     start=True, stop=True)
                    st4 = st4p.tile([128, 512], F16, tag="st4")
                    nc.vector.tensor_scalar(out=st4[:, :ne], in0=dtp[:, :ne],
                                            scalar1=iota_col[:],
                                            scalar2=None,
                                            op0=mybir.AluOpType.is_equal)
                    # gather 4 tiles worth of table rows
                    g4 = g4p.tile([128, 4 * ROW16], F16, tag="g4")
                    for k in range(nt):
                        col = b * TPB + t0 + k
                        nc.gpsimd.indirect_dma_start(
                            out=g4[:, k * ROW16:(k + 1) * ROW16],
                            out_offset=None,
                            in_=table[:],
                            in_offset=bass.IndirectOffsetOnAxis(
                                ap=src_sb[:, col:col + 1], axis=0),
                        )
                    # er per edge via one-hot matmul
                    erp = er_ps.tile([128, 4 * 2 * H], F32, tag="erp")
                    for k in range(nt):
                        nc.tensor.matmul(
                            out=erp[:, k * 2 * H:(k + 1) * 2 * H],
                            lhsT=st4[:, k * 128:(k + 1) * 128],
                            rhs=er_sb[:, b * 2 * H:(b + 1) * 2 * H],
                            start=True, stop=True)
                    # e = el + er ; w = exp(lrelu(e))
                    wsb = wp.tile([128, 4 * H], F32, tag="w")
                    el_view = (g4[:].rearrange("p (t c) -> p t c", c=ROW16)
                               [:, 0:nt, IN_DIM:ROW16].bitcast(F32))
                    erp_v = erp[:, :nt * 2 * H].rearrange("p (t u) -> p t u", u=2 * H)
                    w_v = wsb[:, :nt * H].rearrange("p (t h) -> p t h", h=H)
                    nc.vector.tensor_tensor(
                        out=w_v, in0=el_view, in1=erp_v[:, :, 0:H],
                        op=mybir.AluOpType.add)
                    nc.vector.tensor_tensor(
                        out=w_v, in0=w_v, in1=erp_v[:, :, H:2 * H],
                        op=mybir.AluOpType.add)
                    w5 = wp.tile([128, 4 * H], F32, tag="w5")
                    nc.vector.tensor_scalar(out=w5[:, :nt * H],
                                            in0=wsb[:, :nt * H],
                                            scalar1=NEG_SLOPE, scalar2=None,
                                            op0=mybir.AluOpType.mult)
                    nc.vector.tensor_tensor(out=wsb[:, :nt * H],
                                            in0=wsb[:, :nt * H],
                                            in1=w5[:, :nt * H],
                                            op=mybir.AluOpType.max)
                    nc.scalar.activation(wsb[:, :nt * H], wsb[:, :nt * H],
                                         mybir.ActivationFunctionType.Exp)
                    # w into fp16 slots 128:136; scale messages
                    g4r = g4[:].rearrange("p (t c) -> p t c", c=ROW16)
                    w16_view = g4r[:, 0:nt, IN_DIM:IN_DIM + H]
                    nc.scalar.activation(
                        w16_view,
                        wsb[:, :nt * H].rearrange("p (t h) -> p t h", h=H),
                        mybir.ActivationFunctionType.Copy)
                    feat_view = g4r[:, 0:nt, 0:IN_DIM]
                    w_b = (w16_view
                           .rearrange("p t (h one) -> p t h one", h=H, one=1)
                           .to_broadcast([128, nt, H, D]))
                    nc.vector.tensor_tensor(
                        out=feat_view.rearrange("p t (h d) -> p t h d", d=D),
                        in0=feat_view.rearrange("p t (h d) -> p t h d", d=D),
                        in1=w_b,
                        op=mybir.AluOpType.mult)
                    # per-tile one-hot S + scatter matmul
                    for k in range(nt):
                        col = b * TPB + t0 + k
                        s_sb = sp.tile([128, 128], F16, tag="s")
                        nc.vector.tensor_scalar(
                            out=s_sb[:], in0=iota_row[:],
                            scalar1=dslot_sb[:, col:col + 1],
                            scalar2=None,
                            op0=mybir.AluOpType.is_equal)
                        ti = t0 + k
                        nc.tensor.matmul(out=psb[:],
                                         lhsT=s_sb[:],
                                         rhs=g4[:, k * ROW16:k * ROW16 + ROW],
                                         start=(ti == 0), stop=(ti == TPB - 1))
                # ---- block finalize ----
                ssum = finp.tile([128, H], F32, tag="ssum")
                nc.vector.tensor_scalar(out=ssum[:], in0=psb[:, IN_DIM:ROW],
                                        scalar1=1e-30, scalar2=None,
                                        op0=mybir.AluOpType.add)
                rec = finp.tile([128, H], F32, tag="rec")
                nc.vector.reciprocal(out=rec[:], in_=ssum[:])
                h_sb = finp.tile([128, HD], F32, tag="h")
                rec_b = (rec[:].rearrange("p (h one) -> p h one", h=H, one=1)
                         .to_broadcast([128, H, D]))
                nc.vector.tensor_tensor(
                    out=h_sb[:].rearrange("p (h d) -> p h d", d=D),
                    in0=psb[:, 0:IN_DIM].rearrange("p (h d) -> p h d", d=D),
                    in1=rec_b, op=mybir.AluOpType.mult)
                h16 = finp.tile([128, HD], F16, tag="h16")
                nc.vector.tensor_copy(out=h16[:], in_=h_sb[:])
                sq_sb = finp.tile([128, HD], F16, tag="sq")
                nc.scalar.activation(sq_sb[:], h_sb[:],
                                     mybir.ActivationFunctionType.Square)
                nc.tensor.matmul(out=s1_ps[:], lhsT=h16[:], rhs=ones_col16[:],
                                 start=(b == 0), stop=(b == NBLK - 1))
                nc.tensor.matmul(out=s2_ps[:], lhsT=sq_sb[:], rhs=ones_col16[:],
                                 start=(b == 0), stop=(b == NBLK - 1))
                nc.sync.dma_start(out=h_out[b * 128:(b + 1) * 128, :], in_=h_sb[:])

            stat_sb = constp.tile([128, 2], F32)
            nc.vector.tensor_copy(out=stat_sb[:, 0:1], in_=s1_ps[:])
            nc.vector.tensor_copy(out=stat_sb[:, 1:2], in_=s2_ps[:])
            nc.sync.dma_start(out=st_out[:], in_=stat_sb[:])
            st_scope.__exit__(None, None, None)
            dt_scope.__exit__(None, None, None)
            erp_scope.__exit__(None, None, None)
            blk_scope.__exit__(None, None, None)

    nc.compile()
    return nc


def _build_launch2():
    nc = bacc.Bacc("TRN2", target_bir_lowering=False, debug=False,
                   num_devices=NCORES)
    h_in = nc.dram_tensor("h_in", [SLOTS, HD], F32, kind="ExternalInput")
    xTp = nc.dram_tensor("xTp", [128, SLOTS], F32, kind="ExternalInput")
    ac = nc.dram_tensor("ac", [128, 2], F32, kind="ExternalInput")
    out_t = nc.dram_tensor("out_t", [128, SLOTS], F32, kind="ExternalOutput")

    CH = 512
    NCH = SLOTS // CH  # 24.5 -> handle 24 full + 1 tail of 256
    chunks = [(i * CH, CH) for i in range(NCH)]
    if SLOTS % CH:
        chunks.append((NCH * CH, SLOTS % CH))

    with tile.TileContext(nc) as tc:
        with (
            tc.tile_pool(name="const", bufs=1) as constp,
            tc.tile_pool(name="ld", bufs=4) as ldp,
            tc.tile_pool(name="ps", bufs=3, space="PSUM") as psp,
            tc.tile_pool(name="wk", bufs=3) as wkp,
        ):
            from concourse.masks import make_identity
            ident = constp.tile([128, 128], F32)
            make_identity(nc, ident[:])
            ac_sb = constp.tile([128, 2], F32)
            nc.sync.dma_start(out=ac_sb[:], in_=ac[:])

            for (o, w) in chunks:
                nk = w // 128
                hp = psp.tile([128, CH], F32, tag="hp")
                for k in range(nk):
                    hl = ldp.tile([128, 128], F32, tag="hl")
                    nc.sync.dma_start(
                        out=hl[:], in_=h_in[o + k * 128:o + (k + 1) * 128, :])
                    nc.tensor.transpose(out=hp[:, k * 128:(k + 1) * 128],
                                        in_=hl[:], identity=ident[:])
                h2 = wkp.tile([128, CH], F32, tag="h2")
                nc.vector.tensor_scalar(out=h2[:, :w], in0=hp[:, :w],
                                        scalar1=ac_sb[:, 0:1],
                                        scalar2=ac_sb[:, 1:2],
                                        op0=mybir.AluOpType.mult,
                                        op1=mybir.AluOpType.add)
                m = wkp.tile([128, CH], F32, tag="m")
                nc.vector.tensor_scalar(out=m[:, :w], in0=h2[:, :w],
                                        scalar1=0.0, scalar2=None,
                                        op0=mybir.AluOpType.min)
                nc.scalar.activation(m[:, :w], m[:, :w],
                                     mybir.ActivationFunctionType.Exp)
                nc.vector.tensor_scalar(out=m[:, :w], in0=m[:, :w],
                                        scalar1=-1.0, scalar2=None,
                                        op0=mybir.AluOpType.add)
                # elu = max(h2, exp(min(h2,0))-1)
                nc.vector.tensor_tensor(out=h2[:, :w], in0=h2[:, :w],
                                        in1=m[:, :w],
                                        op=mybir.AluOpType.max)
                xt = ldp.tile([128, CH], F32, tag="xt")
                nc.sync.dma_start(out=xt[:, :w], in_=xTp[:, o:o + w])
                nc.vector.tensor_tensor(out=h2[:, :w], in0=h2[:, :w],
                                        in1=xt[:, :w], op=mybir.AluOpType.add)
                nc.sync.dma_start(out=out_t[:, o:o + w], in_=h2[:, :w])

    nc.compile()
    return nc


def _host_prep(x, src, dst):
    """Shard + balance + pad. Returns per-core index arrays and perms."""
    import heapq
    per_core = []
    for c in range(NCORES):
        lo = c * SLOTS
        hi = min((c + 1) * SLOTS, N)
        nodes_c = hi - lo
        m = (dst >= lo) & (dst < hi)
        e_src = src[m].astype(np.int64)
        e_dstl = (dst[m] - lo).astype(np.int64)
        deg = np.bincount(e_dstl, minlength=nodes_c)
        order = np.argsort(-deg, kind="stable")
        # greedy balance: assign node to least-loaded block with a free slot
        heap = [(0, b) for b in range(NBLK)]
        heapq.heapify(heap)
        slots_used = np.zeros(NBLK, np.int64)
        blk_of = np.empty(nodes_c, np.int64)
        slot_of = np.empty(nodes_c, np.int64)
        spill = []
        for v in order:
            while True:
                load, b = heapq.heappop(heap)
                if slots_used[b] < 128:
                    break
                spill.append((load, b))
            blk_of[v] = b
            slot_of[v] = slots_used[b]
            slots_used[b] += 1
            heapq.heappush(heap, (load + int(deg[v]), b))
        eb = blk_of[e_dstl]
        cap = TPB * 128
        cnt = np.bincount(eb, minlength=NBLK)
        assert cnt.max() <= cap, f"block overflow {cnt.max()} > {cap}"
        eorder = np.argsort(eb, kind="stable")
        offs = np.zeros(NBLK + 1, np.int64)
        np.cumsum(cnt, out=offs[1:])
        within = np.arange(len(eb)) - offs[eb[eorder]]
        p_src = np.full((NBLK, cap), SENT, np.int32)
        p_slot = np.full((NBLK, cap), 300.0, np.float32)
        p_src[eb[eorder], within] = e_src[eorder].astype(np.int32)
        p_slot[eb[eorder], within] = slot_of[e_dstl[eorder]].astype(np.float32)
        # node index per slot (-1 for pad slots)
        node_of_slot = np.full(SLOTS, -1, np.int64)
        node_of_slot[blk_of * 128 + slot_of] = np.arange(nodes_c) + lo
        src_arr = p_src.reshape(NBLK, TPB, 128).transpose(2, 0, 1).reshape(128, TILES)
        dslot_arr = p_slot.reshape(NBLK, TPB, 128).transpose(2, 0, 1).reshape(128, TILES)
        drow_arr = p_slot.reshape(1, EDGES_PAD).astype(np.float16)
        per_core.append((src_arr, dslot_arr, drow_arr, node_of_slot))
    return per_core


def kernel(x, src, dst, W, attn_l, attn_r, bias, gamma, beta):
    global LAST_EXEC_NS
    x = np.asarray(x, np.float32)
    src = np.asarray(src, np.int32)
    dst = np.asarray(dst, np.int32)
    W = np.asarray(W, np.float32)
    attn_l = np.asarray(attn_l, np.float32)
    attn_r = np.asarray(attn_r, np.float32)
    gamma = np.asarray(gamma, np.float32)
    beta = np.asarray(beta, np.float32)

    if "l1" not in _cache:
        _cache["l1"] = _build_launch1()
    if "l2" not in _cache:
        _cache["l2"] = _build_launch2()
    nc1, nc2 = _cache["l1"], _cache["l2"]

    per_core = _host_prep(x, src, dst)

    xT_full = np.zeros((128, NTOT), np.float32)
    xT_full[:, :N] = x.T
    xT_hi = xT_full.astype(np.float16)
    xT_lo = (xT_full - xT_hi.astype(np.float32)).astype(np.float16)
    am = np.zeros((HD, 2 * H), np.float32)
    for h in range(H):
        am[h * D:(h + 1) * D, h] = attn_l[h]
        am[h * D:(h + 1) * D, H + h] = attn_r[h]
    iota_r = np.tile(np.arange(128, dtype=np.float16), (128, 1))
    iota_c = np.arange(128, dtype=np.float32).reshape(128, 1)

    in_maps = []
    xTp_list = []
    for c in range(NCORES):
        src_arr, dslot_arr, drow_arr, node_of_slot = per_core[c]
        xTp = np.zeros((128, SLOTS), np.float32)
        real = node_of_slot >= 0
        xTp[:, real] = x[node_of_slot[real]].T
        xTp_list.append(xTp)
        in_maps.append({
            "xTh": xT_hi, "xTl": xT_lo, "xTp": xTp, "W": W, "am": am,
            "iota_r": iota_r, "iota_c": iota_c,
            "srci": src_arr, "dslot": dslot_arr, "drow": drow_arr,
        })

    res1 = run_bass_kernel_spmd(nc1, in_maps, list(range(NCORES)),
                                **_trace_kwargs())
    LAST_EXEC_NS[0] = res1.exec_time_ns or 0

    # host: combine BN stats (2x128 floats per core)
    S1 = np.zeros(128, np.float64)
    S2 = np.zeros(128, np.float64)
    for c in range(NCORES):
        st = res1.results[c]["st_out"]
        S1 += st[:, 0]
        S2 += st[:, 1]
    mu = (S1 / N).astype(np.float32)
    var = (S2 / N - (S1 / N) ** 2).astype(np.float32)
    a = gamma / np.sqrt(var + EPS)
    cc = beta - a * mu
    ac = np.stack([a, cc], axis=1).astype(np.float32)

    in_maps2 = []
    for c in range(NCORES):
        in_maps2.append({
            "h_in": res1.results[c]["h_out"],
            "xTp": xTp_list[c],
            "ac": ac,
        })
    res2 = run_bass_kernel_spmd(nc2, in_maps2, list(range(NCORES)),
                                **_trace_kwargs())
    LAST_EXEC_NS[1] = res2.exec_time_ns or 0

    out = np.zeros((N, IN_DIM), np.float32)
    for c in range(NCORES):
        node_of_slot = per_core[c][3]
        real = node_of_slot >= 0
        ot = res2.results[c]["out_t"]  # [128, SLOTS]
        out[node_of_slot[real]] = ot[:, real].T
    return out


def _trace_kwargs():
    import os
    if os.environ.get("GAT_TRACE", "0") == "1":
        return {"trace": True}
    return {}
